# revision 1
# baseline (speedup 1.0000x reference)
"""Trainium2 Bass kernel for CustomAttention (dense transformer block).

Full inputs -> full output. Tensor-parallel over heads across 8 NeuronCores:
core c owns heads [4c, 4c+4) i.e. projection columns [512c, 512c+512).
Each core computes q/k/v projections for its heads (RoPE applied on-chip),
causal attention (softmax without max-subtraction; scores bounded ~19), and
a partial output projection over its 512-wide slice of the contraction dim.
The host sums the 8 partials.

All matmuls run as float32r (TF32-like, 1 cycle/row at N>=256).
"""

import math
import numpy as np

import concourse.bass as bass
import concourse.tile as tile
from concourse import bacc, mybir
from concourse.bass_utils import run_bass_kernel_spmd

F32 = mybir.dt.float32
F32R = mybir.dt.float32r
EXPFN = mybir.ActivationFunctionType.Exp

D = 4096          # model dim
H = 32            # heads (total)
HD = 128          # head dim
NCORES = 8
HPC = H // NCORES  # heads per core = 4
MS = HPC * HD      # per-core projection slice = 512
B = 2
S = 2048
T = B * S         # 4096 tokens
SCALE = HD ** -0.5

_compiled = {}


def _build():
    nc = bacc.Bacc("TRN2", target_bir_lowering=False, debug=False,
                   num_devices=NCORES)

    # ---- I/O -------------------------------------------------------------
    xT_d = nc.dram_tensor("xT", [D, T], F32R, kind="ExternalInput").ap()
    wqT_d = nc.dram_tensor("wqT", [D, MS], F32R, kind="ExternalInput").ap()
    wkT_d = nc.dram_tensor("wkT", [D, MS], F32R, kind="ExternalInput").ap()
    wvT_d = nc.dram_tensor("wvT", [D, MS], F32R, kind="ExternalInput").ap()
    woT_d = nc.dram_tensor("woT", [MS, D], F32R, kind="ExternalInput").ap()
    cos_d = nc.dram_tensor("cosT", [HD, S], F32, kind="ExternalInput").ap()
    ssin_d = nc.dram_tensor("ssinT", [HD, S], F32, kind="ExternalInput").ap()
    hmask_d = nc.dram_tensor("hmask", [128, 896], F32, kind="ExternalInput").ap()
    out_d = nc.dram_tensor("outp", [T, D], F32, kind="ExternalOutput").ap()

    # ---- scratch ---------------------------------------------------------
    qT_s = nc.dram_tensor("qT_s", [MS, T], F32R, kind="Internal").ap()
    kT_s = nc.dram_tensor("kT_s", [MS, T], F32R, kind="Internal").ap()
    v_s = nc.dram_tensor("v_s", [T, MS], F32R, kind="Internal").ap()

    with tile.TileContext(nc) as tc:
        _emit(nc, tc, xT_d, wqT_d, wkT_d, wvT_d, woT_d, cos_d, ssin_d,
              hmask_d, out_d, qT_s, kT_s, v_s)

    nc.compile()
    return nc


def _emit(nc, tc, xT_d, wqT_d, wkT_d, wvT_d, woT_d, cos_d, ssin_d,
          hmask_d, out_d, qT_s, kT_s, v_s):
    from contextlib import ExitStack

    TB = 512                 # token block for projections
    NTB = T // TB            # 8
    DT = D // 128            # 32 contraction tiles

    with ExitStack() as ctx:
        const_pool = ctx.enter_context(tc.tile_pool(name="const", bufs=1))

        # rope tables + causal mask, resident
        cos_sb = const_pool.tile([HD, S], F32)
        ssin_sb = const_pool.tile([HD, S], F32)
        hmask_sb = const_pool.tile([128, 896], F32)
        for c in range(4):
            sl = bass.ts(c, S // 4)
            nc.sync.dma_start(cos_sb[:, sl], cos_d[:, sl])
            nc.sync.dma_start(ssin_sb[:, sl], ssin_d[:, sl])
        nc.sync.dma_start(hmask_sb[:], hmask_d[:])
        ones_f = const_pool.tile([128, 128], F32)
        nc.vector.memset(ones_f[:], 1.0)
        ones_sb = const_pool.tile([128, 128], F32R)
        nc.vector.tensor_copy(ones_sb[:], ones_f[:])

        # ================= phase 1a: q/k projections + rope ================
        with ExitStack() as p1:
            wq_pool = p1.enter_context(tc.tile_pool(name="wqk", bufs=1))
            x_pool = p1.enter_context(tc.tile_pool(name="x1a", bufs=10))
            ps_pool = p1.enter_context(
                tc.tile_pool(name="ps1a", bufs=8, space="PSUM"))
            rp_pool = p1.enter_context(tc.tile_pool(name="rope", bufs=6))
            sp_pool = p1.enter_context(tc.tile_pool(name="spill", bufs=6))

            wq_sb = wq_pool.tile([128, DT, MS], F32R)
            wk_sb = wq_pool.tile([128, DT, MS], F32R)
            for g in range(8):  # 4 dt per DMA chunk
                sl = slice(g * 4 * 128, (g + 1) * 4 * 128)
                nc.sync.dma_start(
                    wq_sb[:, g * 4:(g + 1) * 4, :],
                    wqT_d[sl, :].rearrange("(dt p) m -> p dt m", p=128))
                nc.sync.dma_start(
                    wk_sb[:, g * 4:(g + 1) * 4, :],
                    wkT_d[sl, :].rearrange("(dt p) m -> p dt m", p=128))

            for tb in range(NTB):
                tsl = bass.ts(tb, TB)
                # 8 psum accumulation groups (q x 4 m-tiles, k x 4 m-tiles),
                # dt loop outermost so each x tile is consumed immediately
                pss = [ps_pool.tile([128, TB], F32, tag="ps1a", name=f"ps1a_{_g}") for _g in range(2 * HPC)]
                for dt in range(DT):
                    xt = x_pool.tile([128, TB], F32R, tag="x1a")
                    nc.sync.dma_start(
                        xt[:], xT_d[dt * 128:(dt + 1) * 128, tsl])
                    for pi, w_sb in enumerate((wq_sb, wk_sb)):
                        for mt in range(HPC):
                            nc.tensor.matmul(
                                pss[pi * HPC + mt][:],
                                w_sb[:, dt, mt * 128:(mt + 1) * 128],
                                xt[:],
                                start=(dt == 0), stop=(dt == DT - 1))
                # position slice within the sequence for rope tables
                psl = slice((tb * TB) % S, (tb * TB) % S + TB)
                for pi, dst in enumerate((qT_s, kT_s)):
                    for mt in range(HPC):
                        ps = pss[pi * HPC + mt]
                        raw = rp_pool.tile([128, TB], F32, tag="raw", bufs=3)
                        nc.scalar.copy(raw[:], ps[:])
                        # rotate-half operand: partitions swapped by 64
                        sw = rp_pool.tile([128, TB], F32, tag="sw", bufs=3)
                        nc.sync.dma_start(sw[0:64, :], raw[64:128, :])
                        nc.sync.dma_start(sw[64:128, :], raw[0:64, :])
                        qc = rp_pool.tile([128, TB], F32, tag="qc", bufs=2)
                        nc.vector.tensor_mul(qc[:], raw[:], cos_sb[:, psl])
                        qs = rp_pool.tile([128, TB], F32, tag="qs", bufs=2)
                        nc.vector.tensor_mul(qs[:], sw[:], ssin_sb[:, psl])
                        rot = sp_pool.tile([128, TB], F32R, tag="rot", bufs=4)
                        nc.vector.tensor_add(rot[:], qc[:], qs[:])
                        nc.sync.dma_start(
                            dst[mt * 128:(mt + 1) * 128, tsl], rot[:])

        # ================= phase 1b: v projection ==========================
        with ExitStack() as p1b:
            wv_pool = p1b.enter_context(tc.tile_pool(name="wv", bufs=1))
            x_pool = p1b.enter_context(tc.tile_pool(name="x1b", bufs=12))
            ps_pool = p1b.enter_context(
                tc.tile_pool(name="ps1b", bufs=8, space="PSUM"))
            vs_pool = p1b.enter_context(tc.tile_pool(name="vsb", bufs=6))

            wv_sb = wv_pool.tile([128, DT, MS], F32R)
            for g in range(8):
                sl = slice(g * 4 * 128, (g + 1) * 4 * 128)
                nc.sync.dma_start(
                    wv_sb[:, g * 4:(g + 1) * 4, :],
                    wvT_d[sl, :].rearrange("(dt p) m -> p dt m", p=128))

            for tb in range(NTB):
                tsl = bass.ts(tb, TB)
                pss = [ps_pool.tile([128, MS], F32, tag="ps1b", name=f"ps1b_{_g}") for _g in range(TB // 128)]
                for dt in range(DT):
                    xt = x_pool.tile([128, TB], F32R, tag="x1b")
                    nc.sync.dma_start(
                        xt[:], xT_d[dt * 128:(dt + 1) * 128, tsl])
                    for tt in range(TB // 128):
                        nc.tensor.matmul(
                            pss[tt][:],
                            xt[:, tt * 128:(tt + 1) * 128],
                            wv_sb[:, dt, :],
                            start=(dt == 0), stop=(dt == DT - 1))
                for tt in range(TB // 128):
                    vsb = vs_pool.tile([128, MS], F32R, tag="vsb")
                    nc.scalar.copy(vsb[:], pss[tt][:])
                    row = tb * TB + tt * 128
                    nc.sync.dma_start(v_s[row:row + 128, :], vsb[:])

        # ================= phase 2: attention + output proj ================
        with ExitStack() as p2:
            wo_pool = p2.enter_context(tc.tile_pool(name="wo", bufs=1))
            kv_pool = p2.enter_context(tc.tile_pool(name="kv", bufs=2))
            q_pool = p2.enter_context(tc.tile_pool(name="q2", bufs=3))
            e_pool = p2.enter_context(tc.tile_pool(name="expt", bufs=4))
            ctx_pool = p2.enter_context(tc.tile_pool(name="ctx", bufs=4))
            n_pool = p2.enter_context(tc.tile_pool(name="norm", bufs=4))
            o_pool = p2.enter_context(tc.tile_pool(name="osb", bufs=6))
            s_ps_pool = p2.enter_context(
                tc.tile_pool(name="sps", bufs=2, space="PSUM"))
            a_ps_pool = p2.enter_context(
                tc.tile_pool(name="aps", bufs=2, space="PSUM"))
            o_ps_pool = p2.enter_context(
                tc.tile_pool(name="ops", bufs=2, space="PSUM"))

            wo_sb = wo_pool.tile([128, HPC, D], F32R)
            for g in range(4):
                nc.sync.dma_start(
                    wo_sb[:, g, :],
                    woT_d[g * 128:(g + 1) * 128, :])

            IT = 512               # i-tile (query) width
            NIT = S // IT          # 4 per batch

            for b in range(2):
                ctx_tiles = []
                for h in range(HPC):
                    kt = kv_pool.tile([128, S], F32R, tag="k")
                    for c in range(4):
                        sl = bass.ts(c, S // 4)
                        nc.sync.dma_start(
                            kt[:, sl],
                            kT_s[h * 128:(h + 1) * 128,
                                 b * S + c * (S // 4):
                                 b * S + (c + 1) * (S // 4)])
                    vt = kv_pool.tile([128, S // 128, 128], F32R, tag="v")
                    for c in range(4):
                        rows = slice(b * S + c * (S // 4),
                                     b * S + (c + 1) * (S // 4))
                        nc.sync.dma_start(
                            vt[:, c * 4:(c + 1) * 4, :],
                            v_s[rows, h * 128:(h + 1) * 128]
                            .rearrange("(jt p) c -> p jt c", p=128))

                    ctx_h = ctx_pool.tile([128, S], F32R, tag="ctx")
                    ctx_tiles.append(ctx_h)

                    for i in range(NIT):
                        qt = q_pool.tile([128, IT], F32R, tag="q")
                        nc.sync.dma_start(
                            qt[:],
                            qT_s[h * 128:(h + 1) * 128,
                                 b * S + i * IT: b * S + (i + 1) * IT])
                        ctx_ps = a_ps_pool.tile([128, IT], F32, tag="ctxps")
                        rs_ps = a_ps_pool.tile([128, IT], F32, tag="rsps")
                        njt = (i + 1) * IT // 128
                        for jt in range(njt):
                            s_ps = s_ps_pool.tile([128, IT], F32, tag="sps")
                            nc.tensor.matmul(
                                s_ps[:], kt[:, jt * 128:(jt + 1) * 128],
                                qt[:], start=True, stop=True)
                            doff = jt * 128 - i * IT
                            if doff < 0:
                                et = e_pool.tile([128, IT], F32R, tag="et")
                                nc.scalar.activation(et[:], s_ps[:], EXPFN,
                                                     scale=SCALE)
                            else:
                                ef = e_pool.tile([128, IT], F32, tag="ef")
                                nc.scalar.activation(ef[:], s_ps[:], EXPFN,
                                                     scale=SCALE)
                                et = e_pool.tile([128, IT], F32R, tag="et")
                                nc.vector.tensor_mul(
                                    et[:], ef[:],
                                    hmask_sb[:, 384 - doff: 896 - doff])
                            nc.tensor.matmul(
                                ctx_ps[:], vt[:, jt, :], et[:],
                                start=(jt == 0), stop=(jt == njt - 1))
                            nc.tensor.matmul(
                                rs_ps[:], ones_sb[:], et[:],
                                start=(jt == 0), stop=(jt == njt - 1))
                        recip = n_pool.tile([128, IT], F32, tag="recip")
                        nc.vector.reciprocal(recip[:], rs_ps[:])
                        nc.vector.tensor_mul(
                            ctx_h[:, i * IT:(i + 1) * IT],
                            ctx_ps[:], recip[:])

                # output projection for this batch
                for tt in range(S // 128):
                    for et_i in range(D // 512):
                        o_ps = o_ps_pool.tile([128, 512], F32, tag="ops")
                        for h in range(HPC):
                            nc.tensor.matmul(
                                o_ps[:],
                                ctx_tiles[h][:, tt * 128:(tt + 1) * 128],
                                wo_sb[:, h, et_i * 512:(et_i + 1) * 512],
                                start=(h == 0), stop=(h == HPC - 1))
                        osb = o_pool.tile([128, 512], F32, tag="osb")
                        nc.scalar.copy(osb[:], o_ps[:])
                        row = b * S + tt * 128
                        nc.sync.dma_start(
                            out_d[row:row + 128,
                                  et_i * 512:(et_i + 1) * 512], osb[:])


def _host_prep(x, Wq, Wk, Wv, Wo):
    x = np.asarray(x, dtype=np.float32)
    Wq = np.asarray(Wq, dtype=np.float32)
    Wk = np.asarray(Wk, dtype=np.float32)
    Wv = np.asarray(Wv, dtype=np.float32)
    Wo = np.asarray(Wo, dtype=np.float32)

    xT = np.ascontiguousarray(x.reshape(T, D).T)               # [D, T]

    # per-core column slices of W.T  -> [ncores][D, MS]
    def col_shards(W):
        WT = np.ascontiguousarray(
            W.T.reshape(D, NCORES, MS).transpose(1, 0, 2))
        return WT
    wqT = col_shards(Wq)
    wkT = col_shards(Wk)
    wvT = col_shards(Wv)
    # per-core row slices of Wo.T -> [ncores][MS, D]
    woT = np.ascontiguousarray(Wo.T.reshape(NCORES, MS, D))

    # rope tables in [hd, s] layout, matching the reference's fp32 math
    inv = (1.0 / (10000.0 ** (np.arange(0, HD, 2, dtype=np.float32) / HD))
           ).astype(np.float32)
    t = np.arange(S, dtype=np.float32)
    freqs = np.outer(t, inv).astype(np.float32)                # [S, 64]
    cos = np.cos(freqs).T                                      # [64, S]
    sin = np.sin(freqs).T
    cosT = np.ascontiguousarray(
        np.concatenate([cos, cos], axis=0), dtype=np.float32)  # [128, S]
    ssinT = np.ascontiguousarray(
        np.concatenate([-sin, sin], axis=0), dtype=np.float32)

    # causal mask table: hmask[dj, y] = 1 if dj <= y - 384
    dj = np.arange(128)[:, None]
    y = np.arange(896)[None, :]
    hmask = (dj <= y - 384).astype(np.float32)

    return xT, wqT, wkT, wvT, woT, cosT, ssinT, hmask


def kernel(x, mask, Wq, Wk, Wv, Wo, _trace=False):
    del mask  # causal mask is hardcoded (tril), matching the reference
    xT, wqT, wkT, wvT, woT, cosT, ssinT, hmask = _host_prep(x, Wq, Wk, Wv, Wo)

    if "nc" not in _compiled:
        _compiled["nc"] = _build()
    nc = _compiled["nc"]

    in_maps = []
    for c in range(NCORES):
        in_maps.append({
            "xT": xT,
            "wqT": np.ascontiguousarray(wqT[c]),
            "wkT": np.ascontiguousarray(wkT[c]),
            "wvT": np.ascontiguousarray(wvT[c]),
            "woT": np.ascontiguousarray(woT[c]),
            "cosT": cosT,
            "ssinT": ssinT,
            "hmask": hmask,
        })

    res = run_bass_kernel_spmd(nc, in_maps, core_ids=list(range(NCORES)),
                               trace=_trace)

    acc = res.results[0]["outp"].astype(np.float64)
    for c in range(1, NCORES):
        acc += res.results[c]["outp"]
    out = acc.astype(np.float32).reshape(B, S, D)
    if _trace:
        kernel.last_exec_time_ns = res.exec_time_ns
        kernel.last_results = res
    return out



# revision 4
# speedup vs baseline: 1.2637x; 1.2637x over previous
"""Trainium2 Bass kernel for CustomAttention (dense transformer block).

Full inputs -> full output. Tensor-parallel over heads across 8 NeuronCores:
core c owns heads [4c, 4c+4) i.e. projection columns [512c, 512c+512).
Each core computes q/k/v projections for its heads (RoPE applied on-chip),
causal attention (softmax without max-subtraction; scores bounded ~19), and
a partial output projection over its 512-wide slice of the contraction dim.
The host sums the 8 bf16 partials in fp32.

All matmul inputs are bf16 (PSUM accumulation stays fp32). k and v stay
resident in SBUF (no DRAM round trip); q spills to DRAM in bf16. Weights
stream in chunks so the PE starts within ~8us, and wv/wo prefetch during
earlier phases so phase transitions have no PE gap. Output-projection of
batch 0 is interleaved with attention of batch 1 to keep the PE fed while
the scalar engine does exp.
"""

import numpy as np
import ml_dtypes

import concourse.bass as bass
import concourse.tile as tile
from concourse import bacc, mybir
from concourse.bass_utils import run_bass_kernel_spmd

F32 = mybir.dt.float32
BF16 = mybir.dt.bfloat16
EXPFN = mybir.ActivationFunctionType.Exp

D = 4096          # model dim
H = 32            # heads (total)
HD = 128          # head dim
NCORES = 8
HPC = H // NCORES  # heads per core = 4
MS = HPC * HD      # per-core projection slice = 512
B = 2
S = 2048
T = B * S         # 4096 tokens
SCALE = HD ** -0.5
TB = 512          # token block for projections
NTB = T // TB     # 8
DT = D // 128     # 32 contraction tiles
IT = 512          # attention i-tile (query) width
NIT = S // IT     # 4

_compiled = {}


def _build():
    nc = bacc.Bacc("TRN2", target_bir_lowering=False, debug=False,
                   num_devices=NCORES)

    xT_d = nc.dram_tensor("xT", [D, T], BF16, kind="ExternalInput").ap()
    wqT_d = nc.dram_tensor("wqT", [D, MS], BF16, kind="ExternalInput").ap()
    wkT_d = nc.dram_tensor("wkT", [D, MS], BF16, kind="ExternalInput").ap()
    wvT_d = nc.dram_tensor("wvT", [D, MS], BF16, kind="ExternalInput").ap()
    woT_d = nc.dram_tensor("woT", [MS, D], BF16, kind="ExternalInput").ap()
    cos_d = nc.dram_tensor("cosT", [HD, S], BF16, kind="ExternalInput").ap()
    ssin_d = nc.dram_tensor("ssinT", [HD, S], BF16, kind="ExternalInput").ap()
    hmask_d = nc.dram_tensor("hmask", [128, 896], BF16,
                             kind="ExternalInput").ap()
    out_d = nc.dram_tensor("outp", [T, D], BF16, kind="ExternalOutput").ap()

    qT_s = nc.dram_tensor("qT_s", [MS, T], BF16, kind="Internal").ap()

    with tile.TileContext(nc) as tc:
        _emit(nc, tc, xT_d, wqT_d, wkT_d, wvT_d, woT_d, cos_d, ssin_d,
              hmask_d, out_d, qT_s)

    nc.compile()
    return nc


def _emit(nc, tc, xT_d, wqT_d, wkT_d, wvT_d, woT_d, cos_d, ssin_d,
          hmask_d, out_d, qT_s):
    from contextlib import ExitStack

    with ExitStack() as ctx:
        const_pool = ctx.enter_context(tc.tile_pool(name="const", bufs=1))
        x_pool = ctx.enter_context(tc.tile_pool(name="xp", bufs=12))

        # resident k/v (per-core heads, both batches), bf16
        k_sb = const_pool.tile([128, HPC, T], BF16)
        v_sb = const_pool.tile([128, T // 128, MS], BF16)

        # rope tables + causal mask + ones, resident
        cos_sb = const_pool.tile([HD, S], BF16)
        ssin_sb = const_pool.tile([HD, S], BF16)
        hmask_sb = const_pool.tile([128, 896], BF16)
        ones_f = const_pool.tile([128, 128], F32)
        ones_sb = const_pool.tile([128, 128], BF16)

        # ============ phase qk: q/k projections + rope =====================
        with ExitStack() as p1:
            w_pool = p1.enter_context(tc.tile_pool(name="wqk", bufs=1))
            ps_pool = p1.enter_context(
                tc.tile_pool(name="ps1", bufs=8, space="PSUM"))
            rp_pool = p1.enter_context(tc.tile_pool(name="rope", bufs=6))

            # chunked weight tiles: MMs for dt group g start once chunk g is
            # in, so the PE ramps ~8us into the kernel instead of ~60us.
            wq_ch = [w_pool.tile([128, 4, MS], BF16, name=f"wq{g}")
                     for g in range(8)]
            wk_ch = [w_pool.tile([128, 4, MS], BF16, name=f"wk{g}")
                     for g in range(8)]
            for g in range(8):
                sl = slice(g * 4 * 128, (g + 1) * 4 * 128)
                nc.sync.dma_start(
                    wq_ch[g][:],
                    wqT_d[sl, :].rearrange("(dt p) m -> p dt m", p=128))
                nc.sync.dma_start(
                    wk_ch[g][:],
                    wkT_d[sl, :].rearrange("(dt p) m -> p dt m", p=128))

            for c in range(4):
                sl = bass.ts(c, S // 4)
                nc.sync.dma_start(cos_sb[:, sl], cos_d[:, sl])
                nc.sync.dma_start(ssin_sb[:, sl], ssin_d[:, sl])
            nc.sync.dma_start(hmask_sb[:], hmask_d[:])
            nc.vector.memset(ones_f[:], 1.0)
            nc.vector.tensor_copy(ones_sb[:], ones_f[:])

            for tb in range(NTB):
                tsl = bass.ts(tb, TB)
                pss = [ps_pool.tile([128, TB], F32, tag="ps1",
                                    name=f"ps1_{g}") for g in range(2 * HPC)]
                for dt in range(DT):
                    g, r = dt // 4, dt % 4
                    xt = x_pool.tile([128, TB], BF16, tag="x")
                    nc.sync.dma_start(
                        xt[:], xT_d[dt * 128:(dt + 1) * 128, tsl])
                    for mt in range(HPC):
                        nc.tensor.matmul(
                            pss[mt][:],
                            wq_ch[g][:, r, mt * 128:(mt + 1) * 128],
                            xt[:], start=(dt == 0), stop=(dt == DT - 1))
                    for mt in range(HPC):
                        nc.tensor.matmul(
                            pss[HPC + mt][:],
                            wk_ch[g][:, r, mt * 128:(mt + 1) * 128],
                            xt[:], start=(dt == 0), stop=(dt == DT - 1))
                psl = slice((tb * TB) % S, (tb * TB) % S + TB)
                for pi in range(2):
                    for mt in range(HPC):
                        ps = pss[pi * HPC + mt]
                        raw = rp_pool.tile([128, TB], BF16, tag="raw", bufs=3)
                        nc.scalar.copy(raw[:], ps[:])
                        sw = rp_pool.tile([128, TB], BF16, tag="sw", bufs=3)
                        nc.sync.dma_start(sw[0:64, :], raw[64:128, :])
                        nc.sync.dma_start(sw[64:128, :], raw[0:64, :])
                        qc = rp_pool.tile([128, TB], BF16, tag="qc", bufs=2)
                        nc.vector.tensor_mul(qc[:], raw[:], cos_sb[:, psl])
                        qs = rp_pool.tile([128, TB], BF16, tag="qs", bufs=2)
                        nc.vector.tensor_mul(qs[:], sw[:], ssin_sb[:, psl])
                        if pi == 0:
                            rot = rp_pool.tile([128, TB], BF16, tag="rot",
                                               bufs=4)
                            nc.vector.tensor_add(rot[:], qc[:], qs[:])
                            nc.sync.dma_start(
                                qT_s[mt * 128:(mt + 1) * 128, tsl], rot[:])
                        else:
                            nc.vector.tensor_add(
                                k_sb[:, mt, tsl], qc[:], qs[:])

        # ============ phase v: v projection ================================
        with ExitStack() as pv:
            wv_pool = pv.enter_context(tc.tile_pool(name="wv", bufs=1))
            vs_pool = pv.enter_context(tc.tile_pool(name="vsb", bufs=6))
            vps_pool = pv.enter_context(
                tc.tile_pool(name="psv", bufs=8, space="PSUM"))

            wv_ch = [wv_pool.tile([128, 4, MS], BF16, name=f"wv{g}")
                     for g in range(8)]
            for g in range(8):
                sl = slice(g * 4 * 128, (g + 1) * 4 * 128)
                nc.sync.dma_start(
                    wv_ch[g][:],
                    wvT_d[sl, :].rearrange("(dt p) m -> p dt m", p=128))
            # wo prefetch (used in attention phase)
            wo_sb = const_pool.tile([128, HPC, D], BF16)
            for g in range(HPC):
                nc.sync.dma_start(
                    wo_sb[:, g, :], woT_d[g * 128:(g + 1) * 128, :])

            for tb in range(NTB):
                tsl = bass.ts(tb, TB)
                pss = [vps_pool.tile([128, MS], F32, tag="psv",
                                     name=f"psv_{g}") for g in range(4)]
                for dt in range(DT):
                    g, r = dt // 4, dt % 4
                    xt = x_pool.tile([128, TB], BF16, tag="x")
                    nc.sync.dma_start(
                        xt[:], xT_d[dt * 128:(dt + 1) * 128, tsl])
                    for tt in range(4):
                        nc.tensor.matmul(
                            pss[tt][:],
                            xt[:, tt * 128:(tt + 1) * 128],
                            wv_ch[g][:, r, :],
                            start=(dt == 0), stop=(dt == DT - 1))
                for tt in range(4):
                    nc.scalar.copy(v_sb[:, tb * 4 + tt, :], pss[tt][:])

        # ============ phase attn: attention + output projection ============
        with ExitStack() as p2:
            q_pool = p2.enter_context(tc.tile_pool(name="q2", bufs=4))
            e_pool = p2.enter_context(tc.tile_pool(name="expt", bufs=6))
            ctx_pool = p2.enter_context(tc.tile_pool(name="ctx", bufs=8))
            n_pool = p2.enter_context(tc.tile_pool(name="norm", bufs=4))
            o_pool = p2.enter_context(tc.tile_pool(name="osb", bufs=6))
            s_ps_pool = p2.enter_context(
                tc.tile_pool(name="sps", bufs=2, space="PSUM"))
            a_ps_pool = p2.enter_context(
                tc.tile_pool(name="aps", bufs=2, space="PSUM"))
            o_ps_pool = p2.enter_context(
                tc.tile_pool(name="ops", bufs=2, space="PSUM"))

            def attn_head(b, h):
                ctx_h = ctx_pool.tile([128, S], BF16, tag="ctx")
                for i in range(NIT):
                    qt = q_pool.tile([128, IT], BF16, tag="q")
                    nc.sync.dma_start(
                        qt[:],
                        qT_s[h * 128:(h + 1) * 128,
                             b * S + i * IT: b * S + (i + 1) * IT])
                    ctx_ps = a_ps_pool.tile([128, IT], F32, tag="ctxps")
                    rs_ps = a_ps_pool.tile([128, IT], F32, tag="rsps")
                    njt = (i + 1) * IT // 128
                    for jt in range(njt):
                        doff = max(jt * 128 - i * IT, 0)
                        isl = slice(doff, IT)  # valid query range
                        w = IT - doff
                        s_ps = s_ps_pool.tile([128, IT], F32, tag="sps")
                        nc.tensor.matmul(
                            s_ps[:, isl],
                            k_sb[:, h, b * S + jt * 128: b * S + (jt + 1) * 128],
                            qt[:, isl], start=True, stop=True)
                        et = e_pool.tile([128, IT], BF16, tag="et")
                        if jt * 128 - i * IT < 0:
                            nc.scalar.activation(et[:], s_ps[:], EXPFN,
                                                 scale=SCALE)
                        else:
                            ef = e_pool.tile([128, IT], BF16, tag="ef",
                                             bufs=3)
                            nc.scalar.activation(ef[:, isl], s_ps[:, isl],
                                                 EXPFN, scale=SCALE)
                            nc.vector.tensor_mul(
                                et[:, isl], ef[:, isl],
                                hmask_sb[:, 384: 384 + w])
                        nc.tensor.matmul(
                            ctx_ps[:, isl], v_sb[:, b * 16 + jt,
                                                 h * 128:(h + 1) * 128],
                            et[:, isl], start=(jt == 0), stop=(jt == njt - 1))
                        nc.tensor.matmul(
                            rs_ps[:, isl], ones_sb[:], et[:, isl],
                            start=(jt == 0), stop=(jt == njt - 1))
                    recip = n_pool.tile([128, IT], F32, tag="recip")
                    nc.vector.reciprocal(recip[:], rs_ps[:])
                    nc.vector.tensor_mul(
                        ctx_h[:, i * IT:(i + 1) * IT], ctx_ps[:], recip[:])
                return ctx_h

            def outproj_rows(b, ctx_tiles, tts):
                for tt in tts:
                    for e in range(D // 512):
                        o_ps = o_ps_pool.tile([128, 512], F32, tag="ops")
                        for h in range(HPC):
                            nc.tensor.matmul(
                                o_ps[:],
                                ctx_tiles[h][:, tt * 128:(tt + 1) * 128],
                                wo_sb[:, h, e * 512:(e + 1) * 512],
                                start=(h == 0), stop=(h == HPC - 1))
                        osb = o_pool.tile([128, 512], BF16, tag="osb")
                        nc.vector.tensor_copy(osb[:], o_ps[:])
                        row = b * S + tt * 128
                        nc.sync.dma_start(
                            out_d[row:row + 128,
                                  e * 512:(e + 1) * 512], osb[:])

            ctx0 = [attn_head(0, h) for h in range(HPC)]
            ctx1 = []
            # interleave: attention(b=1, head h) with outproj(b=0) rows, so
            # the PE has outproj matmuls to run while ACT does b=1's exps.
            for h in range(HPC):
                ctx1.append(attn_head(1, h))
                outproj_rows(0, ctx0, range(4 * h, 4 * h + 4))
            outproj_rows(1, ctx1, range(16))


def _host_prep(x, Wq, Wk, Wv, Wo):
    bf16 = ml_dtypes.bfloat16
    x = np.asarray(x, dtype=np.float32)

    xT = np.ascontiguousarray(x.reshape(T, D).T).astype(bf16)   # [D, T]

    def col_shards(W):
        W = np.asarray(W, dtype=np.float32)
        return np.ascontiguousarray(
            W.T.reshape(D, NCORES, MS).transpose(1, 0, 2)).astype(bf16)
    wqT = col_shards(Wq)
    wkT = col_shards(Wk)
    wvT = col_shards(Wv)
    woT = np.ascontiguousarray(
        np.asarray(Wo, dtype=np.float32).T.reshape(NCORES, MS, D)
    ).astype(bf16)

    inv = (1.0 / (10000.0 ** (np.arange(0, HD, 2, dtype=np.float32) / HD))
           ).astype(np.float32)
    t = np.arange(S, dtype=np.float32)
    freqs = np.outer(t, inv).astype(np.float32)                # [S, 64]
    cos = np.cos(freqs).T                                      # [64, S]
    sin = np.sin(freqs).T
    cosT = np.concatenate([cos, cos], axis=0).astype(bf16)     # [128, S]
    ssinT = np.concatenate([-sin, sin], axis=0).astype(bf16)

    # causal mask table: hmask[dj, y] = 1 if dj <= y - 384
    dj = np.arange(128)[:, None]
    y = np.arange(896)[None, :]
    hmask = (dj <= y - 384).astype(bf16)

    return xT, wqT, wkT, wvT, woT, cosT, ssinT, hmask


def kernel(x, mask, Wq, Wk, Wv, Wo, _trace=False):
    del mask  # causal mask is hardcoded (tril), matching the reference
    xT, wqT, wkT, wvT, woT, cosT, ssinT, hmask = _host_prep(x, Wq, Wk, Wv, Wo)

    if "nc" not in _compiled:
        _compiled["nc"] = _build()
    nc = _compiled["nc"]

    in_maps = []
    for c in range(NCORES):
        in_maps.append({
            "xT": xT,
            "wqT": np.ascontiguousarray(wqT[c]),
            "wkT": np.ascontiguousarray(wkT[c]),
            "wvT": np.ascontiguousarray(wvT[c]),
            "woT": np.ascontiguousarray(woT[c]),
            "cosT": cosT,
            "ssinT": ssinT,
            "hmask": hmask,
        })

    res = run_bass_kernel_spmd(nc, in_maps, core_ids=list(range(NCORES)),
                               trace=_trace)

    acc = res.results[0]["outp"].astype(np.float32)
    for c in range(1, NCORES):
        acc += res.results[c]["outp"].astype(np.float32)
    out = acc.reshape(B, S, D)
    if _trace:
        kernel.last_exec_time_ns = res.exec_time_ns
        kernel.last_results = res
    return out
